# revision 18
# baseline (speedup 1.0000x reference)
"""Distributed multi-head attention kernel for 8 Trainium2 NeuronCores.

Problem: x[2,2048,768] @ Wqkv[768,2304] + bqkv -> 12-head attention -> @ Wproj + bproj.

Sharding v2: batch (2) x head-group (4 groups of 3 heads) = 8 cores.
Each core computes Q/K/V for ONLY its 3 heads over the full 2048-token
sequence (no redundant K/V compute, no collectives), runs attention for its
heads, and projects through its 192-row slice of Wproj. Each core emits a
PARTIAL y[2048, 768]; the host gather sums the 4 partials per batch and adds
bproj (the TP all-reduce folded into the unshard step).

All matmuls bf16 with full 128-contract / 128-wide stationaries:
- Scores S^T[j,i] per head via KT-pair stationary + per-head zero-padded Q^T
  streaming (zeros kill the other head's K rows). Zero pads come free from
  host-zeroed weight columns.
- Context via the packed-V window: [V_0|1|V_1|1|V_2|1] with a 128-wide
  window per head; the ones-column yields the softmax denominator in psum
  row 64 (even heads) / 63 (odd).
- Softmax runs without max-subtraction (scores are O(1) here); denominators
  are staged to SBUF, gathered via row DMA, inverted with the fast approx
  reciprocal, broadcast back via a bf16 selector matmul, and multiplied into
  ctx^T right before projection (projection is linear -> normalize-late is
  exact).

Exp is the second bottleneck (ScalarE is 1 elem/cycle/lane): optionally a
fraction of the exp groups run on VectorE as a fast-exp (int16 bit-trick:
bf16 bits ~= round(s*184.665*SCALE + 16250.5), one fused tensor_scalar), so
ACT and DVE exponentiate in parallel.

Schedule: 12 pipeline units (3 heads x 4 query chunks of 512). Unit k runs
the score matmuls + exp for unit k while weaving the context matmuls of unit
k-1 one-for-one (ctx work is always ready; scores are exp-paced). QKV
c-tiles fill the lead-in units; the per-chunk normalize+projection chain
runs as soon as head 2's context for that chunk completes.
"""

import numpy as np
import ml_dtypes

B = 2
L = 2048
D = 768
H = 12
HD = 64
SCALE = HD ** -0.5
N_CORES = 8
NH = 3            # heads per core
IC = 4            # query chunks per core
CW = 512          # chunk width
JT = 16           # key tiles of 128
JG = 2            # j-tiles per exp group
VW = 65           # V block width per head (64 ctx + 1 ones)

# exp-group engine split: groups with (g % DVE_EXP_MOD == DVE_EXP_REM) run the
# DVE fast-exp; everything else runs exact exp on ScalarE.  DVE_EXP_MOD=0
# disables the fast-exp path entirely (ACT-only).
DVE_EXP_MOD = 2
DVE_EXP_REM = 1
# bf16-bits fast exp: bits = s_raw*FA + FB  (folds the 1/sqrt(hd) score scale)
FEXP_A = SCALE * 128.0 / float(np.log(2.0))
FEXP_B = (127.0 - 0.043) * 128.0 + 0.5

_CACHED = {}


def _build_nc():
    import concourse.bass as bass
    import concourse.mybir as mybir
    import concourse.tile as tile
    from concourse import bacc

    F32 = mybir.dt.float32
    BF16 = mybir.dt.bfloat16
    I16 = mybir.dt.int16
    Alu = mybir.AluOpType
    Act = mybir.ActivationFunctionType

    nc = bacc.Bacc(target_bir_lowering=False)

    xT_h = nc.declare_dram_parameter("xT", [D, L], BF16, isOutput=False)
    w2_h = nc.declare_dram_parameter("w2", [128, D // 128, 704], BF16, isOutput=False)
    b2_h = nc.declare_dram_parameter("b2", [128, 4], F32, isOutput=False)
    bv2_h = nc.declare_dram_parameter("bv2", [192], F32, isOutput=False)
    wp2_h = nc.declare_dram_parameter("wp2", [128, 2, D], BF16, isOutput=False)
    sel_h = nc.declare_dram_parameter("selm", [NH, 2 * 128], BF16, isOutput=False)
    y_h = nc.declare_dram_parameter("y", [L, D], BF16, isOutput=True)

    DT = D // 128   # 6 contraction tiles

    with tile.TileContext(nc) as tc:
        with tc.tile_pool(name="persist", bufs=1) as pp:
            xT_sb = pp.tile([128, DT, L], BF16)
            w_sb = pp.tile([128, DT, 704], BF16)
            wp_sb = pp.tile([128, 2, D], BF16)
            bias_sb = pp.tile([128, 4], F32)
            bv_sb = pp.tile([128, 192], F32)
            sel_sb = pp.tile([128, 2 * 128], BF16)
            QTz_sb = pp.tile([128, NH, L], BF16)     # per-head Q^T, other half zero
            KT_sb = pp.tile([128, 2, L], BF16)       # tile0 = pair(A,B), tile1 = (C|0)
            V_sb = pp.tile([128, JT, 260], BF16)     # [V_0|1|V_1|1|V_2|1|pad]
            OT2_sb = pp.tile([128, 2, L], BF16)      # ctx^T packed; t1 rows 64:128 zero
            Dall_sb = pp.tile([NH, L], F32)          # denominators (via row DMA)
            Rsb = pp.tile([NH, L], F32)              # 1/denom
            R16 = pp.tile([128, L], BF16)            # bf16 1/denom rows 0:3, rest 0
            stage_sb = pp.tile([128, NH, CW], F32)   # denom row staging (slot = head)

            nc.gpsimd.memset(V_sb, 0.0)
            nc.gpsimd.memset(OT2_sb[64:128, 1, :], 0.0)
            nc.gpsimd.memset(R16, 0.0)
            # zero the complementary halves of the per-head Q^T tiles; the
            # live halves are written by the packed-QT evacuations
            nc.gpsimd.memset(QTz_sb[64:128, 0, :], 0.0)
            nc.gpsimd.memset(QTz_sb[0:64, 1, :], 0.0)
            nc.gpsimd.memset(QTz_sb[64:128, 2, :], 0.0)
            nc.vector.memset(sel_sb, 0.0)
            for h in range(NH):
                nc.vector.memset(V_sb[:, :, h * VW + HD:h * VW + HD + 1], 1.0)

            bv_src = bv2_h[:]
            nc.gpsimd.dma_start(
                out=bv_sb,
                in_=bass.AP(tensor=bv_src.tensor, offset=bv_src.offset,
                            ap=[[0, 128]] + list(bv_src.ap)),
            )
            with (
                tc.tile_pool(name="ptp", bufs=3) as ptp,
                tc.tile_pool(name="ps_s", bufs=2, space="PSUM") as ps_s,
                tc.tile_pool(name="ps_c", bufs=2, space="PSUM") as ps_c,
                tc.tile_pool(name="ps_m", bufs=2, space="PSUM") as ps_m,
                tc.tile_pool(name="yp", bufs=2) as yp,
            ):
                # interleave x / weight c-tile DMAs so the first QKV matmul
                # can start after ~2 tiles instead of the full 4.4 MB
                nc.sync.dma_start(out=bias_sb, in_=b2_h[:])
                nc.sync.dma_start(out=sel_sb[0:NH, :], in_=sel_h[:])
                xT_r = xT_h[:].rearrange("(n p) l -> p n l", p=128)
                for dt in range(DT):
                    for xh in range(2):
                        nc.sync.dma_start(
                            out=xT_sb[:, dt, xh * 1024:(xh + 1) * 1024],
                            in_=xT_r[:, dt, xh * 1024:(xh + 1) * 1024])
                    nc.sync.dma_start(out=w_sb[:, dt, :], in_=w2_h[:, dt, :])
                nc.sync.dma_start(out=wp_sb, in_=wp2_h[:])

                # ---- QKV building blocks ------------------------------------
                # w_sb free-col layout (uniform 128-col blocks):
                # [QTpair | QTc+pad | KTpair | KTc+pad | Wv 512:704]
                def qk_chunk(which, ch):
                    ps = ps_m.tile([128, CW], F32, tag="mps")
                    for dt in range(DT):
                        nc.tensor.matmul(
                            ps,
                            w_sb[:, dt, which * 128:(which + 1) * 128],
                            xT_sb[:, dt, ch * CW:(ch + 1) * CW],
                            start=(dt == 0), stop=(dt == DT - 1),
                        )
                    cs = slice(ch * CW, (ch + 1) * CW)
                    if which == 0:    # packed Q^T pair -> per-head halves
                        nc.vector.tensor_scalar_add(
                            QTz_sb[0:64, 0, cs], ps[0:64, :],
                            bias_sb[0:64, 0:1])
                        nc.vector.tensor_scalar_add(
                            QTz_sb[64:128, 1, cs], ps[64:128, :],
                            bias_sb[64:128, 0:1])
                    elif which == 1:  # Q^T head C (pad half never read)
                        nc.vector.tensor_scalar_add(
                            QTz_sb[0:64, 2, cs], ps[0:64, :],
                            bias_sb[0:64, 1:2])
                    else:             # K^T pair / K^T C
                        nc.vector.tensor_scalar_add(
                            KT_sb[:, which - 2, cs], ps,
                            bias_sb[:, which:which + 1])

                def v_block(lt):
                    ps = ps_m.tile([128, CW], F32, tag="mps")
                    for dt in range(DT):
                        nc.tensor.matmul(
                            ps[:, 0:192],
                            xT_sb[:, dt, lt * 128:(lt + 1) * 128],
                            w_sb[:, dt, 512:704],
                            start=(dt == 0), stop=(dt == DT - 1),
                        )
                    nc.vector.tensor_tensor(
                        V_sb[:, lt, 0:195].rearrange(
                            "p (h c) -> p h c", c=VW)[:, :, 0:HD],
                        ps[:, 0:192].rearrange("p (h d) -> p h d", h=NH),
                        bv_sb.rearrange("p (h d) -> p h d", h=NH),
                        Alu.add,
                    )

                # ---- attention unit: scores+exp for (h,ic), weaving filler --
                def unit(h, ic, prev, filler, pops):
                    # prev = (ph, pic, PT) or None; filler: list of thunks to
                    # sprinkle between score groups (QKV work during lead-in).
                    # pops = thunks to drain per score group; unit 0 must
                    # drain all 16 V blocks (ctx matmuls of later units wait
                    # on them and engine queues execute in order).
                    PT = ptp.tile([128, JT, CW], BF16, tag="PT")
                    kt = h // 2
                    ops = None
                    if prev is not None:
                        ops = ps_c.tile([128, CW], F32, tag="cps")
                    for g in range(JT // JG):
                        sps = ps_s.tile([128, JG, CW], F32, tag="sps")
                        for t in range(JG):
                            jt = JG * g + t
                            if prev is not None:
                                ph, pic, PPT = prev
                                p0 = (ph % 2) * 64
                                voff = ph * VW - p0
                                nc.tensor.matmul(
                                    ops,
                                    V_sb[:, jt, voff:voff + 128],
                                    PPT[:, jt, :],
                                    start=(jt == 0), stop=(jt == JT - 1),
                                    skip_group_check=True,
                                )
                            nc.tensor.matmul(
                                sps[:, t, :],
                                KT_sb[:, kt, jt * 128:(jt + 1) * 128],
                                QTz_sb[:, h, ic * CW:(ic + 1) * CW],
                                start=True, stop=True,
                            )
                        if DVE_EXP_MOD and (g % DVE_EXP_MOD == DVE_EXP_REM):
                            nc.vector.tensor_scalar(
                                PT[:, JG * g:JG * (g + 1), :].bitcast(I16),
                                sps, FEXP_A, FEXP_B, Alu.mult, Alu.add)
                        else:
                            nc.scalar.activation(
                                PT[:, JG * g:JG * (g + 1), :], sps, Act.Exp,
                                scale=SCALE)
                        for _ in range(pops):
                            if filler:
                                filler.pop(0)()
                    if prev is not None:
                        finish_ctx(prev[0], prev[1], ops)
                    return PT

                def finish_ctx(h, ic, ops):
                    p0 = (h % 2) * 64
                    dr = 64 - (h % 2)
                    nc.vector.tensor_copy(
                        OT2_sb[p0:p0 + 64, h // 2, ic * CW:(ic + 1) * CW],
                        ops[p0:p0 + 64, :])
                    # PSUM partition access must start 32-aligned: odd heads'
                    # denom row 63 is read as part of the 32:64 block
                    if h % 2 == 0:
                        nc.vector.tensor_copy(stage_sb[64:65, h, :],
                                              ops[64:65, :])
                    else:
                        nc.vector.tensor_copy(stage_sb[32:64, h, :],
                                              ops[32:64, :])
                    nc.sync.dma_start(
                        out=Dall_sb[h:h + 1, ic * CW:(ic + 1) * CW],
                        in_=stage_sb[dr:dr + 1, h, :])
                    if h == NH - 1:
                        norm_proj(ic)

                def norm_proj(ic):
                    cs = slice(ic * CW, (ic + 1) * CW)
                    nc.vector.reciprocal_approx_fast(
                        out=Rsb[:, cs], in_=Dall_sb[:, cs])
                    nc.vector.tensor_copy(R16[0:NH, cs], Rsb[:, cs])
                    for tx in range(2):
                        rb = ps_m.tile([128, CW], F32, tag="mps")
                        nc.tensor.matmul(
                            rb, sel_sb[:, tx * 128:(tx + 1) * 128],
                            R16[:, cs], start=True, stop=True)
                        nc.vector.tensor_tensor(
                            OT2_sb[:, tx, cs], OT2_sb[:, tx, cs], rb, Alu.mult)
                    # projection for this chunk: 4 i-tiles of 128
                    y_r = y_h[:].rearrange("(n p) e -> p n e", p=128)
                    for it in range(4):
                        git = ic * 4 + it
                        yt = yp.tile([128, D], BF16, tag="yt")
                        for eh in range(2):
                            ps = ps_m.tile([128, CW], F32, tag="mps")
                            for pt in range(2):
                                nc.tensor.matmul(
                                    ps[:, 0:384],
                                    OT2_sb[:, pt, git * 128:(git + 1) * 128],
                                    wp_sb[:, pt, eh * 384:(eh + 1) * 384],
                                    start=(pt == 0), stop=(pt == 1),
                                )
                            if eh == 0:
                                nc.scalar.activation(
                                    yt[:, 0:384], ps[:, 0:384], Act.Copy)
                            else:
                                nc.vector.tensor_copy(
                                    yt[:, 384:768], ps[:, 0:384])
                        nc.sync.dma_start(out=y_r[:, git, :], in_=yt)

                # ---- schedule ----------------------------------------------
                # minimal lead-in: scores (h0, ic0) only need KTpair/QTpair
                # chunk 0; everything else weaves into unit 0 as filler.
                # Ordering constraint: score group g of unit 0 needs KTp
                # chunk g//2, so KTp 1-3 lead the filler list; all 16 V
                # blocks must drain within unit 0 (ctx of unit 1 waits on
                # them in PE program order).
                qk_chunk(2, 0)   # KT pair chunk 0
                qk_chunk(0, 0)   # QT pair chunk 0
                filler = [(lambda ch=ch: qk_chunk(2, ch)) for ch in (1, 2, 3)]
                filler += [(lambda ch=ch: qk_chunk(0, ch)) for ch in (1, 2, 3)]
                filler += [(lambda lt=lt: v_block(lt)) for lt in range(JT)]
                filler += [(lambda ch=ch: qk_chunk(1, ch)) for ch in range(IC)]
                filler += [(lambda ch=ch: qk_chunk(3, ch)) for ch in range(IC)]

                prev = None
                ui = 0
                for h in range(NH):
                    for ic in range(IC):
                        PT = unit(h, ic, prev, filler, 3 if ui == 0 else 1)
                        prev = (h, ic, PT)
                        ui += 1
                # drain remaining filler (shouldn't be any) then final ctx
                for f in filler:
                    f()
                filler.clear()
                ops = ps_c.tile([128, CW], F32, tag="cps")
                ph, pic, PPT = prev
                p0 = (ph % 2) * 64
                voff = ph * VW - p0
                for jt in range(JT):
                    nc.tensor.matmul(
                        ops, V_sb[:, jt, voff:voff + 128], PPT[:, jt, :],
                        start=(jt == 0), stop=(jt == JT - 1),
                        skip_group_check=True,
                    )
                finish_ctx(ph, pic, ops)

    nc.finalize()
    return nc


def _get_nc():
    if "nc" not in _CACHED:
        _CACHED["nc"] = _build_nc()
    return _CACHED["nc"]


def _make_in_maps(x, Wqkv, bqkv, Wproj, bproj):
    bf16 = ml_dtypes.bfloat16
    x = np.asarray(x, dtype=np.float32)
    Wqkv = np.asarray(Wqkv, dtype=np.float32)
    bqkv = np.asarray(bqkv, dtype=np.float32)
    Wproj = np.asarray(Wproj, dtype=np.float32)

    Wq, Wk, Wv = Wqkv[:, 0:D], Wqkv[:, D:2 * D], Wqkv[:, 2 * D:3 * D]
    bq, bk, bv = bqkv[0:D], bqkv[D:2 * D], bqkv[2 * D:3 * D]

    xT = [np.ascontiguousarray(x[b].T.astype(bf16)) for b in range(B)]

    in_maps = []
    for c in range(N_CORES):
        b, hg = c // 4, c % 4
        d0 = hg * 192  # first dim of this core's 3 heads

        # w2 free-col layout per c-tile: [QTpair|QTc+pad|KTpair|KTc+pad|Wv]
        w2 = np.zeros((D, 704), np.float32)
        w2[:, 0:128] = Wq[:, d0:d0 + 128]
        w2[:, 128:192] = Wq[:, d0 + 128:d0 + 192]
        w2[:, 256:384] = Wk[:, d0:d0 + 128]
        w2[:, 384:448] = Wk[:, d0 + 128:d0 + 192]
        w2[:, 512:704] = Wv[:, d0:d0 + 192]
        w2 = np.ascontiguousarray(
            w2.astype(bf16).reshape(D // 128, 128, 704).transpose(1, 0, 2))

        b2 = np.zeros((128, 4), np.float32)
        b2[:, 0] = bq[d0:d0 + 128]
        b2[0:64, 1] = bq[d0 + 128:d0 + 192]
        b2[:, 2] = bk[d0:d0 + 128]
        b2[0:64, 3] = bk[d0 + 128:d0 + 192]

        bv2 = np.ascontiguousarray(bv[d0:d0 + 192])

        wp2 = np.zeros((2, 128, D), np.float32)
        wp2[0] = Wproj[d0:d0 + 128, :]
        wp2[1, 0:64] = Wproj[d0 + 128:d0 + 192, :]
        wp2 = np.ascontiguousarray(wp2.transpose(1, 0, 2).astype(bf16))

        selm = np.zeros((NH, 2 * 128), bf16)
        selm[0, 0:64] = 1.0
        selm[1, 64:128] = 1.0
        selm[2, 128:192] = 1.0

        in_maps.append({
            "xT": xT[b],
            "w2": w2,
            "b2": np.ascontiguousarray(b2),
            "bv2": bv2,
            "wp2": wp2,
            "selm": selm,
        })
    return in_maps


def run(inputs, trace=False):
    """Run the SPMD kernel. Returns (full output [2,2048,768] f32, results)."""
    from concourse.bass_utils import run_bass_kernel_spmd

    nc = _get_nc()
    in_maps = _make_in_maps(**inputs)
    res = run_bass_kernel_spmd(nc, in_maps, list(range(N_CORES)), trace=trace)
    bproj = np.asarray(inputs["bproj"], dtype=np.float32)
    out = np.empty((B, L, D), dtype=np.float32)
    for b in range(B):
        acc = np.zeros((L, D), np.float32)
        for hg in range(4):
            acc += res.results[b * 4 + hg]["y"].astype(np.float32)
        out[b] = acc + bproj
    return out, res


def kernel(**inputs) -> np.ndarray:
    return run(inputs)[0]


# revision 23
# speedup vs baseline: 1.0836x; 1.0836x over previous
"""Distributed multi-head attention kernel for 8 Trainium2 NeuronCores.

Problem: x[2,2048,768] @ Wqkv[768,2304] + bqkv -> 12-head attention -> @ Wproj + bproj.

Sharding v2: batch (2) x head-group (4 groups of 3 heads) = 8 cores.
Each core computes Q/K/V for ONLY its 3 heads over the full 2048-token
sequence (no redundant K/V compute, no collectives), runs attention for its
heads, and projects through its 192-row slice of Wproj. Each core emits a
PARTIAL y[2048, 768]; the host gather sums the 4 partials per batch and adds
bproj (the TP all-reduce folded into the unshard step).

All matmuls bf16 with full 128-contract / 128-wide stationaries:
- Scores S^T[j,i] per head via KT-pair stationary + per-head zero-padded Q^T
  streaming (zeros kill the other head's K rows). Zero pads come free from
  host-zeroed weight columns.
- Context via the packed-V window: [V_0|1|V_1|1|V_2|1] with a 128-wide
  window per head; the ones-column yields the softmax denominator in psum
  row 64 (even heads) / 63 (odd).
- Softmax runs without max-subtraction (scores are O(1) here); denominators
  are staged to SBUF, gathered via row DMA, inverted with the fast approx
  reciprocal, broadcast back via a bf16 selector matmul, and multiplied into
  ctx^T right before projection (projection is linear -> normalize-late is
  exact).

Exp is the second bottleneck (ScalarE is 1 elem/cycle/lane): optionally a
fraction of the exp groups run on VectorE as a fast-exp (int16 bit-trick:
bf16 bits ~= round(s*184.665*SCALE + 16250.5), one fused tensor_scalar), so
ACT and DVE exponentiate in parallel.

Schedule: 12 pipeline units (3 heads x 4 query chunks of 512). Unit k runs
the score matmuls + exp for unit k while weaving the context matmuls of unit
k-1 one-for-one (ctx work is always ready; scores are exp-paced). QKV
c-tiles fill the lead-in units; the per-chunk normalize+projection chain
runs as soon as head 2's context for that chunk completes.
"""

import numpy as np
import ml_dtypes

B = 2
L = 2048
D = 768
H = 12
HD = 64
SCALE = HD ** -0.5
N_CORES = 8
NH = 3            # heads per core
IC = 4            # query chunks per core
CW = 512          # chunk width
JT = 16           # key tiles of 128
JG = 2            # j-tiles per exp group
VW = 65           # V block width per head (64 ctx + 1 ones)

# exp-group engine split: groups in DVE_EXP_GROUPS (of the 8 per unit) run
# the DVE fast-exp; the rest run exact exp on ScalarE.  Empty = ACT-only.
# 2/8 keeps DVE (which also does all the evacuations) level with ScalarE.
DVE_EXP_GROUPS = frozenset((2, 5))
# bf16-bits fast exp: bits = s_raw*FA + FB  (folds the 1/sqrt(hd) score scale)
FEXP_A = SCALE * 128.0 / float(np.log(2.0))
FEXP_B = (127.0 - 0.043) * 128.0 + 0.5

_CACHED = {}


def _build_nc():
    import concourse.bass as bass
    import concourse.mybir as mybir
    import concourse.tile as tile
    from concourse import bacc

    F32 = mybir.dt.float32
    BF16 = mybir.dt.bfloat16
    I16 = mybir.dt.int16
    Alu = mybir.AluOpType
    Act = mybir.ActivationFunctionType

    nc = bacc.Bacc(target_bir_lowering=False)

    xT_h = nc.declare_dram_parameter("xT", [D, L], BF16, isOutput=False)
    w2_h = nc.declare_dram_parameter("w2", [128, D // 128, 704], BF16, isOutput=False)
    b2_h = nc.declare_dram_parameter("b2", [128, 4], F32, isOutput=False)
    bv2_h = nc.declare_dram_parameter("bv2", [192], F32, isOutput=False)
    wp2_h = nc.declare_dram_parameter("wp2", [128, 2, D], BF16, isOutput=False)
    sel_h = nc.declare_dram_parameter("selm", [NH, 2 * 128], BF16, isOutput=False)
    y_h = nc.declare_dram_parameter("y", [L, D], BF16, isOutput=True)

    DT = D // 128   # 6 contraction tiles

    with tile.TileContext(nc) as tc:
        with tc.tile_pool(name="persist", bufs=1) as pp:
            xT_sb = pp.tile([128, DT, L], BF16)
            w_sb = pp.tile([128, DT, 704], BF16)
            wp_sb = pp.tile([128, 2, D], BF16)
            bias_sb = pp.tile([128, 4], F32)
            bv_sb = pp.tile([128, 192], F32)
            sel_sb = pp.tile([128, 2 * 128], BF16)
            QTz_sb = pp.tile([128, NH, L], BF16)     # per-head Q^T, other half zero
            KT_sb = pp.tile([128, 2, L], BF16)       # tile0 = pair(A,B), tile1 = (C|0)
            V_sb = pp.tile([128, JT, 260], BF16)     # [V_0|1|V_1|1|V_2|1|pad]
            OT2_sb = pp.tile([128, 2, L], BF16)      # ctx^T packed; t1 rows 64:128 zero
            Dall_sb = pp.tile([NH, L], F32)          # denominators (via row DMA)
            Rsb = pp.tile([NH, L], F32)              # 1/denom
            R16 = pp.tile([128, L], BF16)            # bf16 1/denom rows 0:3, rest 0
            stage_sb = pp.tile([128, NH, CW], F32)   # denom row staging (slot = head)

            nc.gpsimd.memset(V_sb, 0.0)
            nc.gpsimd.memset(OT2_sb[64:128, 1, :], 0.0)
            nc.gpsimd.memset(R16, 0.0)
            # zero the complementary halves of the per-head Q^T tiles; the
            # live halves are written by the packed-QT evacuations
            nc.gpsimd.memset(QTz_sb[64:128, 0, :], 0.0)
            nc.gpsimd.memset(QTz_sb[0:64, 1, :], 0.0)
            nc.gpsimd.memset(QTz_sb[64:128, 2, :], 0.0)
            nc.vector.memset(sel_sb, 0.0)
            for h in range(NH):
                nc.vector.memset(V_sb[:, :, h * VW + HD:h * VW + HD + 1], 1.0)

            bv_src = bv2_h[:]
            nc.gpsimd.dma_start(
                out=bv_sb,
                in_=bass.AP(tensor=bv_src.tensor, offset=bv_src.offset,
                            ap=[[0, 128]] + list(bv_src.ap)),
            )
            with (
                tc.tile_pool(name="ptp", bufs=3) as ptp,
                tc.tile_pool(name="ps_s", bufs=2, space="PSUM") as ps_s,
                tc.tile_pool(name="ps_c", bufs=2, space="PSUM") as ps_c,
                tc.tile_pool(name="ps_m", bufs=2, space="PSUM") as ps_m,
                tc.tile_pool(name="yp", bufs=2) as yp,
            ):
                # interleave x / weight c-tile DMAs so the first QKV matmul
                # can start after ~2 tiles instead of the full 4.4 MB
                # DMA order matches first-use order: all weight c-tiles, then
                # the first halves of the x c-tiles (enough for QKV chunks
                # 0-1 and V tiles 0-7), then the second halves.
                nc.sync.dma_start(out=bias_sb, in_=b2_h[:])
                nc.sync.dma_start(out=sel_sb[0:NH, :], in_=sel_h[:])
                xT_r = xT_h[:].rearrange("(n p) l -> p n l", p=128)
                for dt in range(DT):
                    nc.sync.dma_start(out=w_sb[:, dt, :], in_=w2_h[:, dt, :])
                for xh in range(2):
                    for dt in range(DT):
                        nc.sync.dma_start(
                            out=xT_sb[:, dt, xh * 1024:(xh + 1) * 1024],
                            in_=xT_r[:, dt, xh * 1024:(xh + 1) * 1024])
                nc.sync.dma_start(out=wp_sb, in_=wp2_h[:])

                # ---- QKV building blocks ------------------------------------
                # w_sb free-col layout (uniform 128-col blocks):
                # [QTpair | QTc+pad | KTpair | KTc+pad | Wv 512:704]
                def qk_chunk(which, ch):
                    ps = ps_m.tile([128, CW], F32, tag="mps")
                    for dt in range(DT):
                        nc.tensor.matmul(
                            ps,
                            w_sb[:, dt, which * 128:(which + 1) * 128],
                            xT_sb[:, dt, ch * CW:(ch + 1) * CW],
                            start=(dt == 0), stop=(dt == DT - 1),
                        )
                    cs = slice(ch * CW, (ch + 1) * CW)
                    if which == 0:    # packed Q^T pair -> per-head halves
                        nc.vector.tensor_scalar_add(
                            QTz_sb[0:64, 0, cs], ps[0:64, :],
                            bias_sb[0:64, 0:1])
                        nc.vector.tensor_scalar_add(
                            QTz_sb[64:128, 1, cs], ps[64:128, :],
                            bias_sb[64:128, 0:1])
                    elif which == 1:  # Q^T head C (pad half never read)
                        nc.vector.tensor_scalar_add(
                            QTz_sb[0:64, 2, cs], ps[0:64, :],
                            bias_sb[0:64, 1:2])
                    else:             # K^T pair / K^T C
                        nc.vector.tensor_scalar_add(
                            KT_sb[:, which - 2, cs], ps,
                            bias_sb[:, which:which + 1])

                def v_block(lt):
                    ps = ps_m.tile([128, CW], F32, tag="mps")
                    for dt in range(DT):
                        nc.tensor.matmul(
                            ps[:, 0:192],
                            xT_sb[:, dt, lt * 128:(lt + 1) * 128],
                            w_sb[:, dt, 512:704],
                            start=(dt == 0), stop=(dt == DT - 1),
                        )
                    nc.vector.tensor_tensor(
                        V_sb[:, lt, 0:195].rearrange(
                            "p (h c) -> p h c", c=VW)[:, :, 0:HD],
                        ps[:, 0:192].rearrange("p (h d) -> p h d", h=NH),
                        bv_sb.rearrange("p (h d) -> p h d", h=NH),
                        Alu.add,
                    )

                # ---- attention unit: scores+exp for (h,ic), weaving filler --
                def unit(h, ic, prev, filler, pops):
                    # prev = (ph, pic, PT) or None; filler: list of thunks to
                    # sprinkle between score groups (QKV work during lead-in).
                    # pops = thunks to drain per score group; unit 0 must
                    # drain all 16 V blocks (ctx matmuls of later units wait
                    # on them and engine queues execute in order).
                    PT = ptp.tile([128, JT, CW], BF16, tag="PT")
                    kt = h // 2
                    ops = None
                    if prev is not None:
                        ops = ps_c.tile([128, CW], F32, tag="cps")
                    for g in range(JT // JG):
                        sps = ps_s.tile([128, JG, CW], F32, tag="sps")
                        for t in range(JG):
                            jt = JG * g + t
                            if prev is not None:
                                ph, pic, PPT = prev
                                p0 = (ph % 2) * 64
                                voff = ph * VW - p0
                                nc.tensor.matmul(
                                    ops,
                                    V_sb[:, jt, voff:voff + 128],
                                    PPT[:, jt, :],
                                    start=(jt == 0), stop=(jt == JT - 1),
                                    skip_group_check=True,
                                )
                            nc.tensor.matmul(
                                sps[:, t, :],
                                KT_sb[:, kt, jt * 128:(jt + 1) * 128],
                                QTz_sb[:, h, ic * CW:(ic + 1) * CW],
                                start=True, stop=True,
                            )
                        if g in DVE_EXP_GROUPS:
                            nc.vector.tensor_scalar(
                                PT[:, JG * g:JG * (g + 1), :].bitcast(I16),
                                sps, FEXP_A, FEXP_B, Alu.mult, Alu.add)
                        else:
                            nc.scalar.activation(
                                PT[:, JG * g:JG * (g + 1), :], sps, Act.Exp,
                                scale=SCALE)
                        for _ in range(pops):
                            if filler:
                                filler.pop(0)()
                    if prev is not None:
                        finish_ctx(prev[0], prev[1], ops)
                    return PT

                def finish_ctx(h, ic, ops):
                    p0 = (h % 2) * 64
                    dr = 64 - (h % 2)
                    nc.vector.tensor_copy(
                        OT2_sb[p0:p0 + 64, h // 2, ic * CW:(ic + 1) * CW],
                        ops[p0:p0 + 64, :])
                    # PSUM partition access must start 32-aligned: odd heads'
                    # denom row 63 is read as part of the 32:64 block
                    if h % 2 == 0:
                        nc.vector.tensor_copy(stage_sb[64:65, h, :],
                                              ops[64:65, :])
                    else:
                        nc.vector.tensor_copy(stage_sb[32:64, h, :],
                                              ops[32:64, :])
                    nc.sync.dma_start(
                        out=Dall_sb[h:h + 1, ic * CW:(ic + 1) * CW],
                        in_=stage_sb[dr:dr + 1, h, :])
                    if h == NH - 1:
                        # reciprocal (DVE-only) inline; the PE pieces of the
                        # normalize+projection chain go onto the filler list
                        # so they weave into the next unit's score groups
                        # instead of stalling the PE queue on DVE latency.
                        cs = slice(ic * CW, (ic + 1) * CW)
                        nc.vector.reciprocal_approx_fast(
                            out=Rsb[:, cs], in_=Dall_sb[:, cs])
                        nc.vector.tensor_copy(R16[0:NH, cs], Rsb[:, cs])
                        for tx in range(2):
                            filler.append(lambda ic=ic, tx=tx: norm_tile(ic, tx))
                        for it in range(4):
                            filler.append(lambda ic=ic, it=it: proj_tile(ic, it))

                def norm_tile(ic, tx):
                    cs = slice(ic * CW, (ic + 1) * CW)
                    rb = ps_m.tile([128, CW], F32, tag="mps")
                    nc.tensor.matmul(
                        rb, sel_sb[:, tx * 128:(tx + 1) * 128],
                        R16[:, cs], start=True, stop=True)
                    nc.vector.tensor_tensor(
                        OT2_sb[:, tx, cs], OT2_sb[:, tx, cs], rb, Alu.mult)

                def proj_tile(ic, it):
                    y_r = y_h[:].rearrange("(n p) e -> p n e", p=128)
                    git = ic * 4 + it
                    yt = yp.tile([128, D], BF16, tag="yt")
                    for eh in range(2):
                        ps = ps_m.tile([128, CW], F32, tag="mps")
                        for pt in range(2):
                            nc.tensor.matmul(
                                ps[:, 0:384],
                                OT2_sb[:, pt, git * 128:(git + 1) * 128],
                                wp_sb[:, pt, eh * 384:(eh + 1) * 384],
                                start=(pt == 0), stop=(pt == 1),
                            )
                        nc.scalar.activation(
                            yt[:, eh * 384:(eh + 1) * 384], ps[:, 0:384],
                            Act.Copy)
                    nc.sync.dma_start(out=y_r[:, git, :], in_=yt)

                # ---- schedule ----------------------------------------------
                # minimal lead-in: scores (h0, ic0) only need KTpair/QTpair
                # chunk 0; everything else weaves into unit 0 as filler.
                # Ordering constraint: score group g of unit 0 needs KTp
                # chunk g//2, so KTp 1-3 lead the filler list; all 16 V
                # blocks must drain within unit 0 (ctx of unit 1 waits on
                # them in PE program order).
                qk_chunk(2, 0)   # KT pair chunk 0
                qk_chunk(0, 0)   # QT pair chunk 0
                filler = [(lambda ch=ch: qk_chunk(2, ch)) for ch in (1, 2, 3)]
                filler += [(lambda ch=ch: qk_chunk(0, ch)) for ch in (1, 2, 3)]
                filler += [(lambda lt=lt: v_block(lt)) for lt in range(JT)]
                filler += [(lambda ch=ch: qk_chunk(1, ch)) for ch in range(IC)]
                filler += [(lambda ch=ch: qk_chunk(3, ch)) for ch in range(IC)]

                prev = None
                ui = 0
                for h in range(NH):
                    for ic in range(IC):
                        PT = unit(h, ic, prev, filler, 3 if ui == 0 else 1)
                        prev = (h, ic, PT)
                        ui += 1
                # drain remaining filler (shouldn't be any) then final ctx
                for f in filler:
                    f()
                filler.clear()
                ops = ps_c.tile([128, CW], F32, tag="cps")
                ph, pic, PPT = prev
                p0 = (ph % 2) * 64
                voff = ph * VW - p0
                for jt in range(JT):
                    nc.tensor.matmul(
                        ops, V_sb[:, jt, voff:voff + 128], PPT[:, jt, :],
                        start=(jt == 0), stop=(jt == JT - 1),
                        skip_group_check=True,
                    )
                finish_ctx(ph, pic, ops)
                # drain the final normalize+projection chain
                while filler:
                    filler.pop(0)()

    nc.finalize()
    return nc


def _get_nc():
    if "nc" not in _CACHED:
        _CACHED["nc"] = _build_nc()
    return _CACHED["nc"]


def _make_in_maps(x, Wqkv, bqkv, Wproj, bproj):
    bf16 = ml_dtypes.bfloat16
    x = np.asarray(x, dtype=np.float32)
    Wqkv = np.asarray(Wqkv, dtype=np.float32)
    bqkv = np.asarray(bqkv, dtype=np.float32)
    Wproj = np.asarray(Wproj, dtype=np.float32)

    Wq, Wk, Wv = Wqkv[:, 0:D], Wqkv[:, D:2 * D], Wqkv[:, 2 * D:3 * D]
    bq, bk, bv = bqkv[0:D], bqkv[D:2 * D], bqkv[2 * D:3 * D]

    xT = [np.ascontiguousarray(x[b].T.astype(bf16)) for b in range(B)]

    in_maps = []
    for c in range(N_CORES):
        b, hg = c // 4, c % 4
        d0 = hg * 192  # first dim of this core's 3 heads

        # w2 free-col layout per c-tile: [QTpair|QTc+pad|KTpair|KTc+pad|Wv]
        w2 = np.zeros((D, 704), np.float32)
        w2[:, 0:128] = Wq[:, d0:d0 + 128]
        w2[:, 128:192] = Wq[:, d0 + 128:d0 + 192]
        w2[:, 256:384] = Wk[:, d0:d0 + 128]
        w2[:, 384:448] = Wk[:, d0 + 128:d0 + 192]
        w2[:, 512:704] = Wv[:, d0:d0 + 192]
        w2 = np.ascontiguousarray(
            w2.astype(bf16).reshape(D // 128, 128, 704).transpose(1, 0, 2))

        b2 = np.zeros((128, 4), np.float32)
        b2[:, 0] = bq[d0:d0 + 128]
        b2[0:64, 1] = bq[d0 + 128:d0 + 192]
        b2[:, 2] = bk[d0:d0 + 128]
        b2[0:64, 3] = bk[d0 + 128:d0 + 192]

        bv2 = np.ascontiguousarray(bv[d0:d0 + 192])

        wp2 = np.zeros((2, 128, D), np.float32)
        wp2[0] = Wproj[d0:d0 + 128, :]
        wp2[1, 0:64] = Wproj[d0 + 128:d0 + 192, :]
        wp2 = np.ascontiguousarray(wp2.transpose(1, 0, 2).astype(bf16))

        selm = np.zeros((NH, 2 * 128), bf16)
        selm[0, 0:64] = 1.0
        selm[1, 64:128] = 1.0
        selm[2, 128:192] = 1.0

        in_maps.append({
            "xT": xT[b],
            "w2": w2,
            "b2": np.ascontiguousarray(b2),
            "bv2": bv2,
            "wp2": wp2,
            "selm": selm,
        })
    return in_maps


def run(inputs, trace=False):
    """Run the SPMD kernel. Returns (full output [2,2048,768] f32, results)."""
    from concourse.bass_utils import run_bass_kernel_spmd

    nc = _get_nc()
    in_maps = _make_in_maps(**inputs)
    res = run_bass_kernel_spmd(nc, in_maps, list(range(N_CORES)), trace=trace)
    bproj = np.asarray(inputs["bproj"], dtype=np.float32)
    out = np.empty((B, L, D), dtype=np.float32)
    for b in range(B):
        acc = np.zeros((L, D), np.float32)
        for hg in range(4):
            acc += res.results[b * 4 + hg]["y"].astype(np.float32)
        out[b] = acc + bproj
    return out, res


def kernel(**inputs) -> np.ndarray:
    return run(inputs)[0]


# revision 25
# speedup vs baseline: 1.2687x; 1.1708x over previous
"""Distributed multi-head attention kernel for 8 Trainium2 NeuronCores.

Problem: x[2,2048,768] @ Wqkv[768,2304] + bqkv -> 12-head attention -> @ Wproj + bproj.

Sharding v2: batch (2) x head-group (4 groups of 3 heads) = 8 cores.
Each core computes Q/K/V for ONLY its 3 heads over the full 2048-token
sequence (no redundant K/V compute, no collectives), runs attention for its
heads, and projects through its 192-row slice of Wproj. Each core emits a
PARTIAL y[2048, 768]; the host gather sums the 4 partials per batch and adds
bproj (the TP all-reduce folded into the unshard step).

All matmuls bf16 with full 128-contract / 128-wide stationaries:
- Scores S^T[j,i] per head via KT-pair stationary + per-head zero-padded Q^T
  streaming (zeros kill the other head's K rows). Zero pads come free from
  host-zeroed weight columns.
- Context via the packed-V window: [V_0|1|V_1|1|V_2|1] with a 128-wide
  window per head; the ones-column yields the softmax denominator in psum
  row 64 (even heads) / 63 (odd).
- Softmax runs without max-subtraction (scores are O(1) here); denominators
  are staged to SBUF, gathered via row DMA, inverted with the fast approx
  reciprocal, broadcast back via a bf16 selector matmul, and multiplied into
  ctx^T right before projection (projection is linear -> normalize-late is
  exact).

Exp is the second bottleneck (ScalarE is 1 elem/cycle/lane): optionally a
fraction of the exp groups run on VectorE as a fast-exp (int16 bit-trick:
bf16 bits ~= round(s*184.665*SCALE + 16250.5), one fused tensor_scalar), so
ACT and DVE exponentiate in parallel.

Schedule: 12 pipeline units (3 heads x 4 query chunks of 512). Unit k runs
the score matmuls + exp for unit k while weaving the context matmuls of unit
k-1 one-for-one (ctx work is always ready; scores are exp-paced). QKV
c-tiles fill the lead-in units; the per-chunk normalize+projection chain
runs as soon as head 2's context for that chunk completes.
"""

import numpy as np
import ml_dtypes

B = 2
L = 2048
D = 768
H = 12
HD = 64
SCALE = HD ** -0.5
N_CORES = 8
NH = 3            # heads per core
IC = 4            # query chunks per core
CW = 512          # chunk width
JT = 16           # key tiles of 128
JG = 2            # j-tiles per exp group
VW = 65           # V block width per head (64 ctx + 1 ones)

# exp-group engine split: groups in DVE_EXP_GROUPS (of the 8 per unit) run
# the DVE fast-exp; the rest run exact exp on ScalarE.  Empty = ACT-only.
# 2/8 keeps DVE (which also does all the evacuations) level with ScalarE.
DVE_EXP_GROUPS = frozenset((2, 5))
# bf16-bits fast exp: bits = s_raw*FA + FB  (folds the 1/sqrt(hd) score scale)
FEXP_A = SCALE * 128.0 / float(np.log(2.0))
FEXP_B = (127.0 - 0.043) * 128.0 + 0.5

_CACHED = {}


def _build_nc():
    import concourse.bass as bass
    import concourse.mybir as mybir
    import concourse.tile as tile
    from concourse import bacc

    F32 = mybir.dt.float32
    BF16 = mybir.dt.bfloat16
    I16 = mybir.dt.int16
    Alu = mybir.AluOpType
    Act = mybir.ActivationFunctionType

    nc = bacc.Bacc(target_bir_lowering=False)

    xT_h = nc.declare_dram_parameter("xT", [D, L], BF16, isOutput=False)
    w2_h = nc.declare_dram_parameter("w2", [128, D // 128, 704], BF16, isOutput=False)
    b2_h = nc.declare_dram_parameter("b2", [128, 4], F32, isOutput=False)
    bv2_h = nc.declare_dram_parameter("bv2", [192], F32, isOutput=False)
    wp2_h = nc.declare_dram_parameter("wp2", [128, 2, D], BF16, isOutput=False)
    sel_h = nc.declare_dram_parameter("selm", [NH, 2 * 128], BF16, isOutput=False)
    y_h = nc.declare_dram_parameter("y", [L, D], BF16, isOutput=True)

    DT = D // 128   # 6 contraction tiles

    with tile.TileContext(nc) as tc:
        with tc.tile_pool(name="persist", bufs=1) as pp:
            xT_sb = pp.tile([128, DT, L], BF16)
            w_sb = pp.tile([128, DT, 704], BF16)
            wp_sb = pp.tile([128, 2, D], BF16)
            bias_sb = pp.tile([128, 4], F32)
            bv_sb = pp.tile([128, 192], F32)
            sel_sb = pp.tile([128, 2 * 128], BF16)
            QTz_sb = pp.tile([128, NH, L], BF16)     # per-head Q^T, other half zero
            KT_sb = pp.tile([128, 2, L], BF16)       # tile0 = pair(A,B), tile1 = (C|0)
            V_sb = pp.tile([128, JT, 260], BF16)     # [V_0|1|V_1|1|V_2|1|pad]
            OT2_sb = pp.tile([128, 2, L], BF16)      # ctx^T packed; t1 rows 64:128 zero
            Dall_sb = pp.tile([NH, L], F32)          # denominators (via row DMA)
            Rsb = pp.tile([NH, L], F32)              # 1/denom
            R16 = pp.tile([128, L], BF16)            # bf16 1/denom rows 0:3, rest 0
            stage_sb = pp.tile([128, NH, CW], F32)   # denom row staging (slot = head)

            nc.gpsimd.memset(V_sb, 0.0)
            nc.gpsimd.memset(OT2_sb[64:128, 1, :], 0.0)
            nc.gpsimd.memset(R16, 0.0)
            # zero the complementary halves of the per-head Q^T tiles; the
            # live halves are written by the packed-QT evacuations
            nc.gpsimd.memset(QTz_sb[64:128, 0, :], 0.0)
            nc.gpsimd.memset(QTz_sb[0:64, 1, :], 0.0)
            nc.gpsimd.memset(QTz_sb[64:128, 2, :], 0.0)
            nc.vector.memset(sel_sb, 0.0)
            for h in range(NH):
                nc.vector.memset(V_sb[:, :, h * VW + HD:h * VW + HD + 1], 1.0)

            bv_src = bv2_h[:]
            nc.gpsimd.dma_start(
                out=bv_sb,
                in_=bass.AP(tensor=bv_src.tensor, offset=bv_src.offset,
                            ap=[[0, 128]] + list(bv_src.ap)),
            )
            with (
                tc.tile_pool(name="ptp", bufs=3) as ptp,
                tc.tile_pool(name="ps_s", bufs=2, space="PSUM") as ps_s,
                tc.tile_pool(name="ps_c", bufs=2, space="PSUM") as ps_c,
                tc.tile_pool(name="ps_m", bufs=2, space="PSUM") as ps_m,
                tc.tile_pool(name="yp", bufs=2) as yp,
            ):
                # interleave x / weight c-tile DMAs so the first QKV matmul
                # can start after ~2 tiles instead of the full 4.4 MB
                # DMA order matches first-use order: all weight c-tiles, then
                # the first halves of the x c-tiles (enough for QKV chunks
                # 0-1 and V tiles 0-7), then the second halves.
                nc.sync.dma_start(out=bias_sb, in_=b2_h[:])
                nc.sync.dma_start(out=sel_sb[0:NH, :], in_=sel_h[:])
                xT_r = xT_h[:].rearrange("(n p) l -> p n l", p=128)
                for dt in range(DT):
                    nc.sync.dma_start(out=w_sb[:, dt, :], in_=w2_h[:, dt, :])
                    nc.sync.dma_start(out=xT_sb[:, dt, 0:1024],
                                      in_=xT_r[:, dt, 0:1024])
                for dt in range(DT):
                    nc.sync.dma_start(out=xT_sb[:, dt, 1024:2048],
                                      in_=xT_r[:, dt, 1024:2048])
                nc.sync.dma_start(out=wp_sb, in_=wp2_h[:])

                # ---- QKV building blocks ------------------------------------
                # w_sb free-col layout (uniform 128-col blocks):
                # [QTpair | QTc+pad | KTpair | KTc+pad | Wv 512:704]
                def qk_chunk(which, ch):
                    ps = ps_m.tile([128, CW], F32, tag="mps")
                    for dt in range(DT):
                        nc.tensor.matmul(
                            ps,
                            w_sb[:, dt, which * 128:(which + 1) * 128],
                            xT_sb[:, dt, ch * CW:(ch + 1) * CW],
                            start=(dt == 0), stop=(dt == DT - 1),
                        )
                    cs = slice(ch * CW, (ch + 1) * CW)
                    if which == 0:    # packed Q^T pair -> per-head halves
                        nc.vector.tensor_scalar_add(
                            QTz_sb[0:64, 0, cs], ps[0:64, :],
                            bias_sb[0:64, 0:1])
                        nc.vector.tensor_scalar_add(
                            QTz_sb[64:128, 1, cs], ps[64:128, :],
                            bias_sb[64:128, 0:1])
                    elif which == 1:  # Q^T head C (pad half never read)
                        nc.vector.tensor_scalar_add(
                            QTz_sb[0:64, 2, cs], ps[0:64, :],
                            bias_sb[0:64, 1:2])
                    else:             # K^T pair / K^T C
                        nc.vector.tensor_scalar_add(
                            KT_sb[:, which - 2, cs], ps,
                            bias_sb[:, which:which + 1])

                def v_block(lt):
                    ps = ps_m.tile([128, CW], F32, tag="mps")
                    for dt in range(DT):
                        nc.tensor.matmul(
                            ps[:, 0:192],
                            xT_sb[:, dt, lt * 128:(lt + 1) * 128],
                            w_sb[:, dt, 512:704],
                            start=(dt == 0), stop=(dt == DT - 1),
                        )
                    nc.vector.tensor_tensor(
                        V_sb[:, lt, 0:195].rearrange(
                            "p (h c) -> p h c", c=VW)[:, :, 0:HD],
                        ps[:, 0:192].rearrange("p (h d) -> p h d", h=NH),
                        bv_sb.rearrange("p (h d) -> p h d", h=NH),
                        Alu.add,
                    )

                # ---- attention unit: scores+exp for (h,ic), weaving filler --
                def unit(h, ic, prev, filler, pops):
                    # prev = (ph, pic, PT) or None; filler: list of thunks to
                    # sprinkle between score groups (QKV work during lead-in).
                    # pops = thunks to drain per score group; unit 0 must
                    # drain all 16 V blocks (ctx matmuls of later units wait
                    # on them and engine queues execute in order).
                    PT = ptp.tile([128, JT, CW], BF16, tag="PT")
                    kt = h // 2
                    ops = None
                    if prev is not None:
                        ops = ps_c.tile([128, CW], F32, tag="cps")
                    for g in range(JT // JG):
                        sps = ps_s.tile([128, JG, CW], F32, tag="sps")
                        for t in range(JG):
                            jt = JG * g + t
                            if prev is not None:
                                ph, pic, PPT = prev
                                p0 = (ph % 2) * 64
                                voff = ph * VW - p0
                                nc.tensor.matmul(
                                    ops,
                                    V_sb[:, jt, voff:voff + 128],
                                    PPT[:, jt, :],
                                    start=(jt == 0), stop=(jt == JT - 1),
                                    skip_group_check=True,
                                )
                            nc.tensor.matmul(
                                sps[:, t, :],
                                KT_sb[:, kt, jt * 128:(jt + 1) * 128],
                                QTz_sb[:, h, ic * CW:(ic + 1) * CW],
                                start=True, stop=True,
                            )
                        if g in DVE_EXP_GROUPS:
                            nc.vector.tensor_scalar(
                                PT[:, JG * g:JG * (g + 1), :].bitcast(I16),
                                sps, FEXP_A, FEXP_B, Alu.mult, Alu.add)
                        else:
                            nc.scalar.activation(
                                PT[:, JG * g:JG * (g + 1), :], sps, Act.Exp,
                                scale=SCALE)
                        for _ in range(pops):
                            if filler:
                                filler.pop(0)()
                    if prev is not None:
                        finish_ctx(prev[0], prev[1], ops)
                    return PT

                def finish_ctx(h, ic, ops):
                    p0 = (h % 2) * 64
                    dr = 64 - (h % 2)
                    nc.vector.tensor_copy(
                        OT2_sb[p0:p0 + 64, h // 2, ic * CW:(ic + 1) * CW],
                        ops[p0:p0 + 64, :])
                    # PSUM partition access must start 32-aligned: odd heads'
                    # denom row 63 is read as part of the 32:64 block
                    if h % 2 == 0:
                        nc.vector.tensor_copy(stage_sb[64:65, h, :],
                                              ops[64:65, :])
                    else:
                        nc.vector.tensor_copy(stage_sb[32:64, h, :],
                                              ops[32:64, :])
                    nc.sync.dma_start(
                        out=Dall_sb[h:h + 1, ic * CW:(ic + 1) * CW],
                        in_=stage_sb[dr:dr + 1, h, :])
                    if h == NH - 1:
                        # reciprocal (DVE-only) inline; the PE pieces of the
                        # normalize+projection chain go onto the filler list
                        # so they weave into the next unit's score groups
                        # instead of stalling the PE queue on DVE latency.
                        cs = slice(ic * CW, (ic + 1) * CW)
                        nc.vector.reciprocal_approx_fast(
                            out=Rsb[:, cs], in_=Dall_sb[:, cs])
                        nc.vector.tensor_copy(R16[0:NH, cs], Rsb[:, cs])
                        for tx in range(2):
                            filler.append(lambda ic=ic, tx=tx: norm_tile(ic, tx))
                        for it in range(4):
                            filler.append(lambda ic=ic, it=it: proj_tile(ic, it))

                def norm_tile(ic, tx):
                    cs = slice(ic * CW, (ic + 1) * CW)
                    rb = ps_m.tile([128, CW], F32, tag="mps")
                    nc.tensor.matmul(
                        rb, sel_sb[:, tx * 128:(tx + 1) * 128],
                        R16[:, cs], start=True, stop=True)
                    nc.vector.tensor_tensor(
                        OT2_sb[:, tx, cs], OT2_sb[:, tx, cs], rb, Alu.mult)

                def proj_tile(ic, it):
                    y_r = y_h[:].rearrange("(n p) e -> p n e", p=128)
                    git = ic * 4 + it
                    yt = yp.tile([128, D], BF16, tag="yt")
                    for eh in range(2):
                        ps = ps_m.tile([128, CW], F32, tag="mps")
                        for pt in range(2):
                            nc.tensor.matmul(
                                ps[:, 0:384],
                                OT2_sb[:, pt, git * 128:(git + 1) * 128],
                                wp_sb[:, pt, eh * 384:(eh + 1) * 384],
                                start=(pt == 0), stop=(pt == 1),
                            )
                        nc.scalar.activation(
                            yt[:, eh * 384:(eh + 1) * 384], ps[:, 0:384],
                            Act.Copy)
                    nc.sync.dma_start(out=y_r[:, git, :], in_=yt)

                # ---- schedule ----------------------------------------------
                # PE clock warm-up: the array only reaches full clock after
                # ~3us of continuous execution, so burn dummy matmuls on a
                # scratch tile while the first DMAs land (results never read)
                warm_sb = pp.tile([128, 64], BF16)
                nc.vector.memset(warm_sb, 0.0)
                wps = ps_m.tile([128, CW], F32, tag="mps")
                for _ in range(40):
                    nc.tensor.matmul(wps[:, 0:64], warm_sb, warm_sb,
                                     start=True, stop=True)

                # minimal lead-in: scores (h0, ic0) only need KTpair/QTpair
                # chunk 0; everything else weaves into unit 0 as filler.
                # Ordering constraint: score group g of unit 0 needs KTp
                # chunk g//2, so KTp 1-3 lead the filler list; all 16 V
                # blocks must drain within unit 0 (ctx of unit 1 waits on
                # them in PE program order).
                qk_chunk(2, 0)   # KT pair chunk 0
                qk_chunk(0, 0)   # QT pair chunk 0
                filler = [(lambda ch=ch: qk_chunk(2, ch)) for ch in (1, 2, 3)]
                filler += [(lambda ch=ch: qk_chunk(0, ch)) for ch in (1, 2, 3)]
                filler += [(lambda lt=lt: v_block(lt)) for lt in range(JT)]
                filler += [(lambda ch=ch: qk_chunk(1, ch)) for ch in range(IC)]
                filler += [(lambda ch=ch: qk_chunk(3, ch)) for ch in range(IC)]

                prev = None
                ui = 0
                for h in range(NH):
                    for ic in range(IC):
                        PT = unit(h, ic, prev, filler, 3 if ui == 0 else 1)
                        prev = (h, ic, PT)
                        ui += 1
                # drain remaining filler (shouldn't be any) then final ctx
                for f in filler:
                    f()
                filler.clear()
                ops = ps_c.tile([128, CW], F32, tag="cps")
                ph, pic, PPT = prev
                p0 = (ph % 2) * 64
                voff = ph * VW - p0
                for jt in range(JT):
                    nc.tensor.matmul(
                        ops, V_sb[:, jt, voff:voff + 128], PPT[:, jt, :],
                        start=(jt == 0), stop=(jt == JT - 1),
                        skip_group_check=True,
                    )
                finish_ctx(ph, pic, ops)
                # drain the final normalize+projection chain
                while filler:
                    filler.pop(0)()

    nc.finalize()
    return nc


def _get_nc():
    if "nc" not in _CACHED:
        _CACHED["nc"] = _build_nc()
    return _CACHED["nc"]


def _make_in_maps(x, Wqkv, bqkv, Wproj, bproj):
    bf16 = ml_dtypes.bfloat16
    x = np.asarray(x, dtype=np.float32)
    Wqkv = np.asarray(Wqkv, dtype=np.float32)
    bqkv = np.asarray(bqkv, dtype=np.float32)
    Wproj = np.asarray(Wproj, dtype=np.float32)

    Wq, Wk, Wv = Wqkv[:, 0:D], Wqkv[:, D:2 * D], Wqkv[:, 2 * D:3 * D]
    bq, bk, bv = bqkv[0:D], bqkv[D:2 * D], bqkv[2 * D:3 * D]

    xT = [np.ascontiguousarray(x[b].T.astype(bf16)) for b in range(B)]

    in_maps = []
    for c in range(N_CORES):
        b, hg = c // 4, c % 4
        d0 = hg * 192  # first dim of this core's 3 heads

        # w2 free-col layout per c-tile: [QTpair|QTc+pad|KTpair|KTc+pad|Wv]
        w2 = np.zeros((D, 704), np.float32)
        w2[:, 0:128] = Wq[:, d0:d0 + 128]
        w2[:, 128:192] = Wq[:, d0 + 128:d0 + 192]
        w2[:, 256:384] = Wk[:, d0:d0 + 128]
        w2[:, 384:448] = Wk[:, d0 + 128:d0 + 192]
        w2[:, 512:704] = Wv[:, d0:d0 + 192]
        w2 = np.ascontiguousarray(
            w2.astype(bf16).reshape(D // 128, 128, 704).transpose(1, 0, 2))

        b2 = np.zeros((128, 4), np.float32)
        b2[:, 0] = bq[d0:d0 + 128]
        b2[0:64, 1] = bq[d0 + 128:d0 + 192]
        b2[:, 2] = bk[d0:d0 + 128]
        b2[0:64, 3] = bk[d0 + 128:d0 + 192]

        bv2 = np.ascontiguousarray(bv[d0:d0 + 192])

        wp2 = np.zeros((2, 128, D), np.float32)
        wp2[0] = Wproj[d0:d0 + 128, :]
        wp2[1, 0:64] = Wproj[d0 + 128:d0 + 192, :]
        wp2 = np.ascontiguousarray(wp2.transpose(1, 0, 2).astype(bf16))

        selm = np.zeros((NH, 2 * 128), bf16)
        selm[0, 0:64] = 1.0
        selm[1, 64:128] = 1.0
        selm[2, 128:192] = 1.0

        in_maps.append({
            "xT": xT[b],
            "w2": w2,
            "b2": np.ascontiguousarray(b2),
            "bv2": bv2,
            "wp2": wp2,
            "selm": selm,
        })
    return in_maps


def run(inputs, trace=False):
    """Run the SPMD kernel. Returns (full output [2,2048,768] f32, results)."""
    from concourse.bass_utils import run_bass_kernel_spmd

    nc = _get_nc()
    in_maps = _make_in_maps(**inputs)
    res = run_bass_kernel_spmd(nc, in_maps, list(range(N_CORES)), trace=trace)
    bproj = np.asarray(inputs["bproj"], dtype=np.float32)
    out = np.empty((B, L, D), dtype=np.float32)
    for b in range(B):
        acc = np.zeros((L, D), np.float32)
        for hg in range(4):
            acc += res.results[b * 4 + hg]["y"].astype(np.float32)
        out[b] = acc + bproj
    return out, res


def kernel(**inputs) -> np.ndarray:
    return run(inputs)[0]
